# revision 27
# baseline (speedup 1.0000x reference)
"""Multi-head attention Trainium2 kernel (Bass/Tile), 8-core SPMD.

Problem: B=2, S=2048, D=1024, H=16 heads of d=64.
Sharding: core c -> batch c//4, 4 heads starting at 4*(c%4).
Each core computes its heads' Q/K/V projections, attention, and the
partial output projection (transposed); host sums the 4 partials per
batch and adds bo.

Device-side layout notes:
  - All activations live transposed ([feature, seq]) so every matmul
    contraction runs over the partition dim.
  - Scores are computed transposed (S^T[sk, sq]) so that P^T = exp(S^T)
    feeds the P@V matmul directly as the moving operand.
  - V carries an extra ones column, so the U^T = V'^T @ P^T matmul also
    emits softmax row-sums for free (row 64 of the psum tile).
  - Matmuls run in float32r (full-rate fp32 on the PE array).
"""

import numpy as np

import concourse.bass as bass
import concourse.mybir as mybir
import concourse.tile as tile
from concourse import bacc
from concourse.bass_utils import run_bass_kernel_spmd

F32 = mybir.dt.float32
F32R = mybir.dt.float32r
BF16 = mybir.dt.bfloat16
AF = mybir.ActivationFunctionType

B, S, D = 2, 2048, 1024
H, DH = 16, 64
NCORES = 8
HL = H // (NCORES // B)       # 4 heads per core
DL = HL * DH                  # 256 local projection dims
PAIRS = HL // 2               # 2 head pairs (packed into 128 partitions)
NKT = D // 128                # 8 contraction tiles for projections
SB = 512                      # seq block (matmul moving-dim chunk)
NSB = S // SB                 # 4
NSKT = S // 128               # 16 key-seq tiles
SCALE = 0.125                 # 1/sqrt(64)

LAST_EXEC_NS = None
_TRACE = False
_TRACE_KW = {}


def _bcast_part(ap, parts):
    """View `ap` with the partition dim replaced by a step-0 broadcast."""
    return bass.AP(tensor=ap.tensor, offset=ap.offset, ap=[[0, parts]] + list(ap.ap[1:]))


def _emit(tc, nc, t):
    import contextlib

    ctx = contextlib.ExitStack()
    with ctx:
        const = ctx.enter_context(tc.tile_pool(name="const", bufs=1))
        persist = ctx.enter_context(tc.tile_pool(name="persist", bufs=1))
        # x chunks are one DMA per (tensor, s-block): [128, 8, 512] bf16, 8KB
        # per partition; 4 bufs = one in use + prefetch headroom.
        xin = ctx.enter_context(tc.tile_pool(name="xin", bufs=4))
        ptp = ctx.enter_context(tc.tile_pool(name="ptp", bufs=2))
        outp = ctx.enter_context(tc.tile_pool(name="outp", bufs=3))
        misc = ctx.enter_context(tc.tile_pool(name="misc", bufs=3))

        # ---------- constants & weights ----------
        # Weight DMAs are split in halves and emitted interleaved with the x
        # chunks that pair with them (inside the projection loops below), so
        # the first matmul starts ~2us in instead of waiting ~14us of
        # serialized weight transfers. Weights and x are bf16: same PE rate
        # as fp32r in this regime but half the HBM traffic.
        wq_sb = const.tile([128, NKT, DL], BF16)
        wk_sb = const.tile([128, NKT, DL], BF16)
        wv_sb = const.tile([128, NKT, DL], BF16)
        wo_sb = const.tile([128, 2, D], BF16)
        bq_sb = const.tile([128, 2], F32)
        nc.sync.dma_start(out=bq_sb, in_=t["bq"].rearrange("(t p) -> p t", p=128))
        bk_sb = const.tile([128, 2], F32)
        nc.sync.dma_start(out=bk_sb, in_=t["bk"].rearrange("(t p) -> p t", p=128))
        bvb = const.tile([128, DL], F32)
        nc.sync.dma_start(out=bvb, in_=_bcast_part(t["bv"].rearrange("(o d) -> o d", o=1), 128))

        # ---------- persistent activations ----------
        qT = persist.tile([128, PAIRS, S], F32R)   # [dh-in-pair, pair, s]
        kT = persist.tile([128, PAIRS, S], F32R)
        v_sb = persist.tile([128, NSKT, HL, DH + 1], F32R)  # [sk, skt, head, d|1]
        aT = persist.tile([128, PAIRS, S], BF16)   # normalized attention, transposed
        # Whole-tile memset (strided/f32r memsets fail walrus ISA checks); the
        # V evacuations overwrite everything except the ones column.
        nc.vector.memset(v_sb.bitcast(F32), 1.0)
        ones_row = persist.tile([1, 128], F32R)    # K=1 broadcast-matmul stationary
        nc.vector.memset(ones_row.bitcast(F32), 1.0)
        # Warm-up exp so the ACT table set loads at t~0 instead of on the
        # first real exp's critical path.
        warm = persist.tile([1, 1], F32)
        nc.scalar.activation(warm, ones_row.bitcast(F32)[0:1, 0:1], AF.Exp)

        # ---------- shared psum pools ----------
        # 8 banks total: pp 2 (projection/outproj/rbc ring), p_big 4 (score
        # tiles), p_ut 2 (U^T accumulators).
        pp = ctx.enter_context(tc.tile_pool(name="pp", bufs=2, space="PSUM"))
        p_big = ctx.enter_context(tc.tile_pool(name="p_big", bufs=2, space="PSUM"))
        p_ut = ctx.enter_context(tc.tile_pool(name="p_ut", bufs=2, space="PSUM"))

        uacc = persist.tile([DH + 1, HL, NSB, SB], F32)  # U^T accumulator (SBUF)

        # out DRAM viewed [mg, p, mi, s]: row (mg*2+mi)*128 + p, col s
        out4 = t["outF"].rearrange("(g m p) s -> g p m s", g=4, m=2)

        def load_w_half(w_sb, name, half):
            nc.sync.dma_start(
                out=w_sb[:, half * 4:(half + 1) * 4, :],
                in_=t[name][half * 512:(half + 1) * 512, :]
                .rearrange("(c p) d -> p c d", p=128))

        def load_x(name, sb, parts=1):
            # one DMA per (tensor, s-block) — split into `parts` kt-slices in
            # the prologue so the first matmuls start ~2us in.
            xt = xin.tile([128, NKT, SB], BF16, name=f"x_{name}_{sb}", tag="x")
            kpp = NKT // parts
            for j in range(parts):
                nc.sync.dma_start(
                    out=xt[:, j * kpp:(j + 1) * kpp, :],
                    in_=t[name][j * kpp * 128:(j + 1) * kpp * 128,
                                sb * SB:(sb + 1) * SB]
                    .rearrange("(c p) s -> p c s", p=128))
            return xt

        # ---------- filler machinery ----------
        # PE work chunks (~400-900ns each) queued up and emitted between the
        # exp and PV of each attention iteration: the attention inner loop is
        # otherwise ACT-bound (exp 1038ns vs 854ns of PE work per iteration),
        # so the projections/epilogue ride in the PE bubbles for free.
        # Chunks carry labels; attention() declares the chunks it depends on
        # via need() (emission-order = program order, so a consumer emitted
        # before its producer would silently read stale data).
        filler = []
        done, pending = set(), set()
        pace = {"rate": 0.0, "credit": 0.0, "iters": 1}

        def pop_one():
            label, fn = filler.pop(0)
            fn()
            if label:
                done.add(label)

        def pump():
            # credit pacing: spread the queue evenly over the remaining
            # attention iterations of this key block instead of draining
            # greedily (an unfilled iteration stalls the PE ~360ns).
            pace["credit"] += pace["rate"]
            pace["iters"] = max(pace["iters"] - 1, 1)
            while filler and pace["credit"] >= 1.0:
                pop_one()
                pace["credit"] -= 1.0

        def set_iters(n):
            pace["iters"] = max(n, 1)
            pace["rate"] = len(filler) / pace["iters"]

        def need(label):
            if label in pending and label not in done:
                while label not in done:
                    pop_one()
                    pace["credit"] = min(pace["credit"] - 1.0, 0.0)

        def add_chunks(chunks):
            for label, fn in chunks:
                if label:
                    pending.add(label)
                filler.append((label, fn))
            pace["rate"] = len(filler) / pace["iters"]

        def qproj_chunks(j):
            # block j of the Q projection: 8 matmul chunks + 1 bias evac
            st = {}

            def mm(kt):
                def go():
                    if kt == 0:
                        st["ps"] = [pp.tile([128, SB], F32, name=f"qps_{j}_{i}", tag="pp")
                                    for i in range(2)]
                        st["x"] = xq_t[j]
                    for dht in range(2):
                        nc.tensor.matmul(st["ps"][dht],
                                         wq_sb[:, kt, dht * 128:(dht + 1) * 128],
                                         st["x"][:, kt, :],
                                         start=(kt == 0), stop=(kt == NKT - 1))
                return go

            def evac():
                for dht in range(2):
                    nc.vector.tensor_scalar_add(qT[:, dht, j * SB:(j + 1) * SB],
                                                st["ps"][dht], bq_sb[:, dht:dht + 1])
            return [(None, mm(kt)) for kt in range(NKT)] + [(("q", j), evac)]

        def kproj_chunks(sb):
            # seq-sliced: two ~427ns chunks per 128-key quarter (8 narrow
            # matmuls each + that slice's bias evac), so chunks for block sb
            # can slide into block sb's own early attention iterations.
            st = {}

            def part(q, dht):
                def go():
                    if q == 0 and dht == 0:
                        st["ps"] = [pp.tile([128, SB], F32, name=f"kps_{sb}_{i}", tag="pp")
                                    for i in range(2)]
                        st["x"] = xk_t[sb]
                    c = slice(q * 128, (q + 1) * 128)
                    for kt in range(NKT):
                        nc.tensor.matmul(st["ps"][dht][:, c],
                                         wk_sb[:, kt, dht * 128:(dht + 1) * 128],
                                         st["x"][:, kt, c],
                                         start=(kt == 0 and q == 0),
                                         stop=(kt == NKT - 1),
                                         skip_group_check=True)
                    nc.vector.tensor_scalar_add(
                        kT[:, dht, sb * SB + q * 128:sb * SB + (q + 1) * 128],
                        st["ps"][dht][:, c], bk_sb[:, dht:dht + 1])
                return go
            return [(("k", sb * 4 + q) if dht == 1 else None, part(q, dht))
                    for q in range(4) for dht in range(2)]

        def vproj_chunks(sb):
            # seq-sliced like K: two ~427ns chunks per 128-key quarter.
            st = {}

            def part(ss, kh):
                def go():
                    if ss == 0 and kh == 0:
                        st["ps"] = [pp.tile([128, SB], F32, name=f"vps_{sb}_{i}", tag="pp")
                                    for i in range(2)]
                        st["x"] = xv_t[sb]
                    half, grp = ss % 2, ss // 2
                    for kt in range(kh * 4, kh * 4 + 4):
                        # two seq-subtiles share one psum bank; only the first
                        # MM in the bank uses start=True
                        nc.tensor.matmul(st["ps"][grp][:, half * DL:(half + 1) * DL],
                                         st["x"][:, kt, ss * 128:(ss + 1) * 128],
                                         wv_sb[:, kt, :],
                                         start=(kt == 0 and half == 0),
                                         stop=(kt == NKT - 1),
                                         skip_group_check=True)
                    if kh == 1:
                        skt = sb * 4 + grp * 2 + half  # == sb*4 + ss
                        nc.vector.tensor_add(
                            v_sb[:, skt, :, 0:DH],
                            st["ps"][grp][:, half * DL:(half + 1) * DL]
                            .rearrange("p (h d) -> p h d", h=HL),
                            bvb.rearrange("p (h d) -> p h d", h=HL))
                return go
            return [(("v", sb * 4 + ss) if kh == 1 else None, part(ss, kh))
                    for ss in range(4) for kh in range(2)]

        def epilogue_norm(isq, pr2):
            # normalize one pair (DVE reads the broadcast reciprocal straight
            # from PSUM); appended right after that pair's U accumulate.
            q0 = isq * SB

            def norm(hi):
                def go():
                    h = pr2 * 2 + hi
                    od = hi * DH
                    # NOTE: reciprocal_approx_fast (custom DVE ucode) returns
                    # garbage on this axon terminal — standard reciprocal only.
                    rinv = misc.tile([1, SB], F32R, name="rinv", tag="rinv")
                    with nc.allow_low_precision(reason="fp32r rounding of 1/rowsum"):
                        nc.vector.reciprocal(rinv, uacc[DH:DH + 1, h, isq, :])
                    rbc1 = pp.tile([128, SB], F32, name="rbc1", tag="pp")
                    nc.tensor.matmul(rbc1, ones_row, rinv,
                                     start=True, stop=True, skip_group_check=True)
                    nc.vector.tensor_mul(aT[od:od + DH, pr2, q0:q0 + SB],
                                         uacc[0:DH, h, isq, :], rbc1[0:DH, :])
                return go
            return [(None, norm(hi)) for hi in range(2)]

        def epilogue_out(isq):
            # outproj + evacuation, one 128-row tile (~427ns) per chunk
            q0 = isq * SB
            st = {}

            def mm(mt):
                def go():
                    op = pp.tile([128, SB], F32, name=f"op_{isq}_{mt}", tag="pp")
                    for jt in range(2):
                        nc.tensor.matmul(op, wo_sb[:, jt, mt * 128:(mt + 1) * 128],
                                         aT[:, jt, q0:q0 + SB],
                                         start=(jt == 0), stop=(jt == 1))
                    st[mt] = op
                return go

            def evac(mt):
                def go():
                    op = st.pop(mt)
                    ot = outp.tile([128, SB], F32, name="ot", tag="ot")
                    # Pool/GPSIMD can't read PSUM on TRN2; split evacuations
                    # between DVE and ACT.
                    if mt % 2 == 0:
                        nc.vector.tensor_copy(ot, op)
                    else:
                        nc.scalar.copy(ot, op)
                    nc.sync.dma_start(out=out4[mt // 2, :, mt % 2, q0:q0 + SB],
                                      in_=ot)
                return go

            out = []
            for mt in range(8):
                out.append((None, mm(mt)))
                out.append((None, evac(mt)))
            return out

        def attention(sb, pr2, isq):
            q0 = isq * SB
            need(("q", isq))
            u2 = [p_ut.tile([DH + 1, SB], F32,
                            name=f"u_{sb}_{pr2}_{isq}_{hi}", tag="ut")
                  for hi in range(2)]
            for skt in range(sb * 4, sb * 4 + 4):
                need(("k", skt))
                stt = p_big.tile([128, 2 * SB], F32, name="stt", tag="big")
                for hi in range(2):
                    od = hi * DH
                    nc.tensor.matmul(stt[:, hi * SB:(hi + 1) * SB],
                                     kT[od:od + DH, pr2, skt * 128:(skt + 1) * 128],
                                     qT[od:od + DH, pr2, q0:q0 + SB],
                                     start=True, stop=True)
                pt = ptp.tile([128, 2 * SB], F32R, name="pt", tag="pt")
                nc.scalar.activation(pt, stt, AF.Exp, scale=SCALE)
                pump()
                need(("v", skt))
                for hi in range(2):
                    h = pr2 * 2 + hi
                    nc.tensor.matmul(u2[hi], v_sb[:, skt, h, :],
                                     pt[:, hi * SB:(hi + 1) * SB],
                                     start=(skt == sb * 4), stop=(skt == sb * 4 + 3))
            for hi in range(2):
                h = pr2 * 2 + hi
                sl = uacc[:, h, isq, :]
                if sb == 0:
                    nc.vector.tensor_copy(sl, u2[hi])
                else:
                    nc.vector.tensor_add(sl, sl, u2[hi])

        # ---------- prologue: Q/K/V projections for block 0 ----------
        xq_t, xk_t, xv_t = {}, {}, {}
        load_w_half(wq_sb, "wqT", 0)
        xq_t[0] = load_x("xqT", 0, parts=4)
        load_w_half(wq_sb, "wqT", 1)
        load_w_half(wk_sb, "wkT", 0)
        load_w_half(wk_sb, "wkT", 1)
        xk_t[0] = load_x("xkT", 0, parts=2)
        load_w_half(wv_sb, "wvT", 0)
        load_w_half(wv_sb, "wvT", 1)
        xv_t[0] = load_x("xvT", 0, parts=2)
        for _, fn in qproj_chunks(0):
            fn()
        for _, fn in kproj_chunks(0):
            fn()
        for _, fn in vproj_chunks(0):
            fn()

        # ---------- main loop: attention with filler-paced projections ----
        for sb in range(NSB):
            if sb == 0:
                # everything the rest of sb0 needs, in deadline order
                for j in range(1, NSB):
                    xq_t[j] = load_x("xqT", j)
                xk_t[1] = load_x("xkT", 1)
                xv_t[1] = load_x("xvT", 1)
                nc.sync.dma_start(out=wo_sb,
                                  in_=t["woT"].rearrange("(c p) m -> p c m", p=128))
                for j in range(1, NSB):
                    add_chunks(qproj_chunks(j))
                add_chunks(kproj_chunks(1))
                add_chunks(vproj_chunks(1))
            elif sb < NSB - 1:
                xk_t[sb + 1] = load_x("xkT", sb + 1)
                xv_t[sb + 1] = load_x("xvT", sb + 1)
                add_chunks(kproj_chunks(sb + 1))
                add_chunks(vproj_chunks(sb + 1))
            set_iters(8 * NSB)
            for isq in range(NSB):
                for pr2 in range(PAIRS):
                    attention(sb, pr2, isq)
                    if sb == NSB - 1:
                        add_chunks(epilogue_norm(isq, pr2))
                if sb == NSB - 1:
                    add_chunks(epilogue_out(isq))
        while filler:
            pop_one()


def build():
    nc = bacc.Bacc("TRN2", target_bir_lowering=False, debug=False, num_devices=NCORES)
    t = {}
    for name, shape in [("xqT", [D, S]), ("xkT", [D, S]), ("xvT", [D, S]),
                        ("wqT", [D, DL]), ("wkT", [D, DL]), ("wvT", [D, DL]),
                        ("woT", [DL, D])]:
        t[name] = nc.dram_tensor(name, shape, BF16, kind="ExternalInput").ap()
    for name, shape in [("bq", [DL]), ("bk", [DL]), ("bv", [DL])]:
        t[name] = nc.dram_tensor(name, shape, F32, kind="ExternalInput").ap()
    t["outF"] = nc.dram_tensor("outF", [D, S], F32, kind="ExternalOutput").ap()
    with tile.TileContext(nc) as tc:
        _emit(tc, nc, t)
    nc.compile()
    return nc


def _bf16(a):
    import ml_dtypes
    return np.ascontiguousarray(np.asarray(a, dtype=np.float32)).astype(ml_dtypes.bfloat16)


def shard(inputs):
    q = np.asarray(inputs["query"], dtype=np.float32)
    k = np.asarray(inputs["key"], dtype=np.float32)
    v = np.asarray(inputs["value"], dtype=np.float32)
    Wq = np.asarray(inputs["Wq"], dtype=np.float32)
    Wk = np.asarray(inputs["Wk"], dtype=np.float32)
    Wv = np.asarray(inputs["Wv"], dtype=np.float32)
    Wo = np.asarray(inputs["Wo"], dtype=np.float32)
    bq = np.asarray(inputs["bq"], dtype=np.float32)
    bk = np.asarray(inputs["bk"], dtype=np.float32)
    bv = np.asarray(inputs["bv"], dtype=np.float32)
    xT = [(_bf16(q[b].T), _bf16(k[b].T), _bf16(v[b].T)) for b in range(B)]
    maps = []
    for c in range(NCORES):
        b, hb = divmod(c, NCORES // B)
        js = slice(hb * DL, (hb + 1) * DL)
        xq, xk, xv = xT[b]
        maps.append({
            "xqT": xq, "xkT": xk, "xvT": xv,
            "wqT": _bf16(Wq[js].T),
            "wkT": _bf16(Wk[js].T),
            "wvT": _bf16(Wv[js].T),
            "woT": _bf16(Wo[:, js].T),
            "bq": np.ascontiguousarray(bq[js]),
            "bk": np.ascontiguousarray(bk[js]),
            "bv": np.ascontiguousarray(bv[js]),
        })
    return maps


def unshard(results, inputs):
    bo = np.asarray(inputs["bo"], dtype=np.float32)
    out = np.empty((B, S, D), np.float32)
    g = NCORES // B
    for b in range(B):
        acc = results[b * g]["outF"].copy()
        for i in range(1, g):
            acc += results[b * g + i]["outF"]
        out[b] = acc.T + bo
    return out


def kernel(**inputs):
    global LAST_EXEC_NS
    nc = build()
    maps = shard(inputs)
    res = run_bass_kernel_spmd(nc, maps, core_ids=list(range(NCORES)),
                               trace=_TRACE, **_TRACE_KW)
    LAST_EXEC_NS = res.exec_time_ns
    return unshard(res.results, inputs)

